# revision 1
# baseline (speedup 1.0000x reference)
"""Trainium2 Bass kernel for ContinuousWaveletLayer (CWT energy).

Reference computation:
  bank = Morlet wavelet bank [32 scales, Lmax=256] (static)
  coef[b,s,t] = 'same' conv of x[b,:] (len 8192) with bank[s,:]
  out[b,s]    = mean_t(coef^2) * softmax(scale_weights)[s]

Device strategy (8 NeuronCores, scale-parallel, 4 scales/core):
  The conv is phrased as Toeplitz matmuls on the tensor engine. With x
  zero-padded (128 left / 128 right) and viewed time-major in blocks of
  128, the output block B (128 time steps) for scale s is

      coef_B[to, b] = sum_{D=0..2} G[s,D].T @ Xblk[B+D]          (K=128)

  where G[s,D][a,to] = g_s[128*D + a - to] (g_s = reversed bank row,
  zero outside [0,256)) and Xblk[A][a,b] = xpad[128*A + a, b].
  x is stored in SBUF as [a=128 partitions, (A,b) free], so the rhs for
  (D, 4-block group) is just a contiguous 512-wide free-dim slice; the
  3 D-matmuls accumulate in PSUM.  Squares are computed on the scalar /
  vector engines (alternating) and accumulated in SBUF; the final
  partition reduction is a ones-vector matmul.  Host applies the final
  1/N and softmax scaling on the gathered [32,128] sums (O(4K) flops).
"""

import os
import sys
from contextlib import ExitStack

import numpy as np

sys.path.insert(0, "/opt/trn_rl_repo")

import concourse.bass as bass
import concourse.mybir as mybir
from concourse import tile
from concourse.bass_utils import run_bass_kernel_spmd
from concourse.vector_clock import ScopedClock


def _drain_and_barrier_single_wait(self, tick_clock, wait_clock):
    """TileContext._drain_and_barrier, but the kernel-tail drain's
    global-clock waits are spread over a chain of single-wait drains —
    the walrus build here allows only one sync wait per instruction."""
    drain_inst = self.nc.sync.drain()
    wait_clock.add_sem_waits(
        drain_inst.ins, ScopedClock({None: tick_clock.global_clock})
    )
    si = drain_inst.ins.sync_info
    waits = list(si.on_wait)
    if len(waits) > 1:
        si.on_wait = [waits[0]]
        sems = {h.name: h for h in self.sems.allocated().values()}
        for w in waits[1:]:
            d2 = self.nc.sync.drain()
            d2.wait_op(sems[w.ant_name], w.wait_value, "sem-ge")
    self.nc.all_engine_barrier()
    assert self.sems is not None
    popped = self.nc._tile_sem_poison_stack.pop()
    assert popped is self._sem_poison
    self.nc.clear_and_free_semaphores(list(self.sems.allocated().values()))
    self.nc.all_engine_barrier()


tile.TileContext._drain_and_barrier = _drain_and_barrier_single_wait

N_CORES = 8
S_TOTAL = 32          # number of scales
S_PER = 4             # scales per core
P = 128               # partition / block size
NT = 8192             # time samples
LMAX = 256            # padded kernel length
NBLK = 66             # input blocks: (128 + 8192 + 128) / 128
NOUT = 64             # output blocks: 8192 / 128
NGRP = 16             # groups of 4 output blocks (N=512 matmuls)
F32 = mybir.dt.float32
BF16 = mybir.dt.bfloat16

LAST_RESULTS = None   # BassKernelResults of the most recent run (for test.py)


def _morlet_kernel_bank(n_scales: int, n: int) -> np.ndarray:
    Lmax = min(8 * n_scales, n)
    bank = np.zeros((n_scales, Lmax), dtype=np.float32)
    for i, s in enumerate(range(1, n_scales + 1)):
        L = min(8 * s, n)
        t = np.linspace(-4.0 * s, 4.0 * s, L)
        w = np.exp(-t**2 / (2.0 * s**2)) * np.cos(5.0 * t / s)
        w = w / np.sqrt(s)
        off = (Lmax - 1) // 2 - (L - 1) // 2
        bank[i, off : off + L] = w.astype(np.float32)
    return bank


def _toeplitz_weights() -> np.ndarray:
    """G[s, D][a, to] = g_s[128*D + a - to], zero outside support."""
    bank = _morlet_kernel_bank(S_TOTAL, NT)          # [32, 256]
    g = bank[:, ::-1].copy()                         # reversed rows
    a = np.arange(P)[:, None]
    to = np.arange(P)[None, :]
    G = np.zeros((S_TOTAL, 3, P, P), dtype=np.float32)
    for D in range(3):
        d = 128 * D + a - to
        valid = (d >= 0) & (d < LMAX)
        dc = np.clip(d, 0, LMAX - 1)
        for s in range(S_TOTAL):
            G[s, D] = np.where(valid, g[s][dc], 0.0)
    return G


GCOLS = S_PER * 3 * P          # 1536 weight columns
XCOLS = NBLK * P               # 8448 x columns


def _build_nc() -> bass.Bass:
    nc = bass.Bass()
    # combined input, one DMA → one semaphore lane for every matmul dep:
    #   xg[:, :GCOLS]    = per-core Toeplitz weights (G[s,D,a,to])
    #   xg[:, GCOLS:-1]  = x time-major: xpad[128*A + a, b]
    #   xg[:, -1]        = ones column (partition reducer)
    xg = nc.dram_tensor("xg", [P, GCOLS + XCOLS + 1], BF16, kind="ExternalInput")
    # per-core partial energies, un-folded: outp[s, (Bsub, b)]; the host
    # sums the 4 column groups (keeps DVE out of the kernel → fewer
    # semaphore procs for the tail drain)
    outp = nc.dram_tensor("outp", [1, S_PER * 512], F32, kind="ExternalOutput")

    with tile.TileContext(nc) as tc, ExitStack() as ctx:
        xpool = ctx.enter_context(tc.tile_pool(name="x", bufs=1))
        # one sq buffer per (ng, s): no slot reuse → no WAR-induced second
        # wait on the ACT squares (walrus allows 1 sync wait/instruction)
        sqpool = ctx.enter_context(tc.tile_pool(name="sq", bufs=NGRP * S_PER))
        # fp32 PSUM-evict scratch for the DVE square path; slot WARs are
        # DVE-vs-DVE (same engine) so reuse costs no extra waits
        cppool = ctx.enter_context(tc.tile_pool(name="cp", bufs=4))
        rowpool = ctx.enter_context(tc.tile_pool(name="row", bufs=4))
        pspool = ctx.enter_context(tc.tile_pool(name="ps", bufs=4, space="PSUM"))
        psepool = ctx.enter_context(tc.tile_pool(name="pse", bufs=1, space="PSUM"))

        xgsb = xpool.tile([P, GCOLS + XCOLS + 1], BF16)
        # one DMA: a single InstDMACopy fans out across all 16 SDMA engines
        nc.sync.dma_start(out=xgsb[:, :], in_=xg[:, :])
        onesb = xgsb[:, GCOLS + XCOLS : GCOLS + XCOLS + 1]

        # per-scale PSUM energy accumulators [1, (Bsub, b)]
        pes = [
            psepool.tile([1, 512], F32, tag=f"pe{s}", name=f"pe{s}")
            for s in range(S_PER)
        ]

        # main conv loop; all cross-engine deps are 1-wait:
        #   conv matmul:  DMA sem (once) / evict-engine sem (bank recycle)
        #   evict+square: PE sem (ACT path) or PE sem + DVE-self (DVE path)
        #   reduce matmul (PE, accumulates into pes[s]): ACT/DVE sem
        for ng in range(NGRP):
            for s in range(S_PER):
                pt = pspool.tile([P, 512], F32)
                for D in range(3):
                    gc = (s * 3 + D) * P
                    xc = GCOLS + (ng * 4 + D) * P
                    lhsT = xgsb[:, gc : gc + P]
                    rhs = xgsb[:, xc : xc + 4 * P]
                    nc.tensor.matmul(
                        pt[:, :], lhsT, rhs, start=(D == 0), stop=(D == 2)
                    )
                sq = sqpool.tile([P, 512], BF16)
                if (ng * S_PER + s) % 2 == 0:
                    # ACT path: square+cast straight out of PSUM
                    nc.scalar.square(sq[:, :], pt[:, :])
                else:
                    # DVE path: fp32 copy out of PSUM, then square+cast
                    cp = cppool.tile([P, 512], F32)
                    nc.vector.tensor_copy(cp[:, :], pt[:, :])
                    nc.vector.tensor_mul(sq[:, :], cp[:, :], cp[:, :])
                nc.tensor.matmul(
                    pes[s][:, :],
                    onesb,
                    sq[:, :],
                    start=(ng == 0),
                    stop=(ng == NGRP - 1),
                )

        # final: evict the [1,512] accumulators side by side on partition 0
        # (engines can only write at partition base 0), single DMA out
        rowout = rowpool.tile([1, S_PER * 512], F32, tag="rowout", name="rowout")
        for s in range(S_PER):
            nc.scalar.copy(rowout[:, s * 512 : (s + 1) * 512], pes[s][:, :])
        nc.sync.dma_start(out=outp[:, :], in_=rowout[:, :])

    return nc


_NC_CACHE = None


def _get_nc() -> bass.Bass:
    global _NC_CACHE
    if _NC_CACHE is None:
        _NC_CACHE = _build_nc()
    return _NC_CACHE


def kernel(x: np.ndarray, scale_weights: np.ndarray, _trace: bool = False) -> np.ndarray:
    global LAST_RESULTS
    x = np.asarray(x, dtype=np.float32)
    scale_weights = np.asarray(scale_weights, dtype=np.float32)
    assert x.shape == (P, NT) and scale_weights.shape == (S_TOTAL,)

    # host prep: zero-pad, transpose to time-major blocked layout
    xpad = np.zeros((NBLK * P, P), dtype=np.float32)
    xpad[P : P + NT, :] = x.T
    # xb2[a, A*128 + b] = xpad[A*128 + a, b]
    xb2 = np.ascontiguousarray(
        xpad.reshape(NBLK, P, P).transpose(1, 0, 2).reshape(P, NBLK * P)
    )

    G = _toeplitz_weights()  # [32, 3, 128, 128]
    # combined per-core input: [weights | x | ones], bf16 for the 1-col/cycle
    # matmul stream; core c handles scales [4c, 4c+4)
    import ml_dtypes

    bf16 = ml_dtypes.bfloat16
    ones = np.ones((P, 1), dtype=np.float32)
    xgs = []
    for c in range(N_CORES):
        Gc = G[c * S_PER : (c + 1) * S_PER].reshape(S_PER * 3, P, P)
        gw2 = Gc.transpose(1, 0, 2).reshape(P, GCOLS)
        xgs.append(
            np.ascontiguousarray(
                np.concatenate([gw2, xb2, ones], axis=1).astype(bf16)
            )
        )

    nc = _get_nc()
    in_maps = [{"xg": xgs[c]} for c in range(N_CORES)]
    res = run_bass_kernel_spmd(nc, in_maps, list(range(N_CORES)), trace=_trace)
    LAST_RESULTS = res

    # gather + unshard: [8 cores][1, 4 scales * (4 Bsub * 128 b)] -> [128, 32]
    esum = np.concatenate(
        [res.results[c]["outp"].reshape(S_PER, 512) for c in range(N_CORES)],
        axis=0,
    )  # [32, 512]
    esum = esum.reshape(S_TOTAL, 4, P).sum(axis=1)  # fold Bsub -> [32, 128]
    energy = esum.T / np.float32(NT)

    w = scale_weights.astype(np.float64)
    e = np.exp(w - w.max())
    sm = (e / e.sum()).astype(np.float32)
    return (energy * sm[None, :]).astype(np.float32)


if __name__ == "__main__":
    rng = np.random.default_rng(0)
    x = rng.standard_normal((P, NT), dtype=np.float32)
    sw = rng.standard_normal(S_TOTAL, dtype=np.float32)
    out = kernel(x, sw)
    print("kernel output shape:", out.shape, out.dtype)



# revision 3
# speedup vs baseline: 1.7741x; 1.7741x over previous
"""Trainium2 Bass kernel for ContinuousWaveletLayer (CWT energy).

Reference computation:
  bank = Morlet wavelet bank [32 scales, Lmax=256] (static)
  coef[b,s,t] = 'same' conv of x[b,:] (len 8192) with bank[s,:]
  out[b,s]    = mean_t(coef^2) * softmax(scale_weights)[s]

Device strategy (8 NeuronCores, scale-parallel, 4 scales/core):
  Toeplitz matmuls on the tensor engine in fp8 (e4m3).  With x padded
  (128/128) and viewed time-major in blocks of 128, output block B for
  scale s is   coef_B = sum_{D=0..2} G[s,D].T @ Xblk[B+D]  (K=128).
  The D=0,1 pair is fused into one DoubleRow matmul (K=256, 2 fp8
  weights per PE cell), D=2 is a normal fp8 matmul accumulating into
  the same PSUM bank.  Squares evacuate PSUM on ACT (scales 0,1 ->
  fp8 direct) and DVE (scales 2,3: bf16 copy + 2x bf16 mul -> fp8).
  Per-scale time-reduction is a DoubleRow "selector" matmul that
  reduces TWO scales' squares at once into a [2,512] PSUM accumulator
  (K=256, accumulated over all 16 block groups).  The input DMA is
  split into 5 chunks so conv starts ~1.5us after the DMA begins, and
  dummy warm-up matmuls keep the PE HAM un-throttled during the DMA.
  Host folds the 4 sub-block columns, applies 1/N and softmax.
"""

import os
import sys
from contextlib import ExitStack

import numpy as np

sys.path.insert(0, "/opt/trn_rl_repo")

import concourse.bass as bass
import concourse.mybir as mybir
from concourse import tile
from concourse.ap import AP
from concourse.bass_utils import run_bass_kernel_spmd
from concourse.vector_clock import ScopedClock


def _drain_and_barrier_single_wait(self, tick_clock, wait_clock):
    """TileContext._drain_and_barrier, but the kernel-tail drain's
    global-clock waits are spread over a chain of single-wait drains —
    the walrus build here allows only one sync wait per instruction."""
    drain_inst = self.nc.sync.drain()
    wait_clock.add_sem_waits(
        drain_inst.ins, ScopedClock({None: tick_clock.global_clock})
    )
    si = drain_inst.ins.sync_info
    waits = list(si.on_wait)
    if len(waits) > 1:
        si.on_wait = [waits[0]]
        sems = {h.name: h for h in self.sems.allocated().values()}
        for w in waits[1:]:
            d2 = self.nc.sync.drain()
            d2.wait_op(sems[w.ant_name], w.wait_value, "sem-ge")
    self.nc.all_engine_barrier()
    assert self.sems is not None
    popped = self.nc._tile_sem_poison_stack.pop()
    assert popped is self._sem_poison
    self.nc.clear_and_free_semaphores(list(self.sems.allocated().values()))
    self.nc.all_engine_barrier()


tile.TileContext._drain_and_barrier = _drain_and_barrier_single_wait

N_CORES = 8
S_TOTAL = 32          # number of scales
S_PER = 4             # scales per core
P = 128               # partition / block size
NT = 8192             # time samples
LMAX = 256            # padded kernel length
NBLK = 66             # input blocks: (128 + 8192 + 128) / 128
NGRP = 16             # groups of 4 output blocks (N=512 matmuls)
F32 = mybir.dt.float32
BF16 = mybir.dt.bfloat16
FP8 = mybir.dt.float8e4
DR = mybir.MatmulPerfMode.DoubleRow

GCOLS = S_PER * 3 * P          # 1536 weight columns
SELBASE = GCOLS                # DoubleRow reduce selector (32 cols)
XBASE = GCOLS + 32
CHUNK_BLKS = 18                # x chunk k holds blocks [16k, 16k+18)
CHUNK_COLS = CHUNK_BLKS * P    # 2304
NCHUNK = 4
COLS = XBASE + NCHUNK * CHUNK_COLS
NWARM = 8                      # PE warm-up matmuls during the input DMA

LAST_RESULTS = None   # BassKernelResults of the most recent run (for test.py)


def _morlet_kernel_bank(n_scales: int, n: int) -> np.ndarray:
    Lmax = min(8 * n_scales, n)
    bank = np.zeros((n_scales, Lmax), dtype=np.float32)
    for i, s in enumerate(range(1, n_scales + 1)):
        L = min(8 * s, n)
        t = np.linspace(-4.0 * s, 4.0 * s, L)
        w = np.exp(-t**2 / (2.0 * s**2)) * np.cos(5.0 * t / s)
        w = w / np.sqrt(s)
        off = (Lmax - 1) // 2 - (L - 1) // 2
        bank[i, off : off + L] = w.astype(np.float32)
    return bank


def _toeplitz_weights() -> np.ndarray:
    """G[s, D][a, to] = g_s[128*D + a - to], zero outside support."""
    bank = _morlet_kernel_bank(S_TOTAL, NT)          # [32, 256]
    g = bank[:, ::-1].copy()                         # reversed rows
    a = np.arange(P)[:, None]
    to = np.arange(P)[None, :]
    G = np.zeros((S_TOTAL, 3, P, P), dtype=np.float32)
    for D in range(3):
        d = 128 * D + a - to
        valid = (d >= 0) & (d < LMAX)
        dc = np.clip(d, 0, LMAX - 1)
        for s in range(S_TOTAL):
            G[s, D] = np.where(valid, g[s][dc], 0.0)
    return G


def _strided(sl, dims):
    """Manual AP on a tile slice: dims = [(stride, n), ...] free dims,
    partition dim inherited from the 2D slice."""
    return AP(
        tensor=sl.tensor,
        offset=sl.offset,
        ap=[list(sl.ap[0])] + [[st, n] for st, n in dims],
    )


def _build_nc() -> bass.Bass:
    nc = bass.Bass()
    xg = nc.dram_tensor("xg", [P, COLS], FP8, kind="ExternalInput")
    # per-core partial energies: [2, (pair, Bsub, b)]; scale = 2*pair + row
    outp = nc.dram_tensor("outp", [2, 1024], F32, kind="ExternalOutput")

    with tile.TileContext(nc) as tc, ExitStack() as ctx:
        xpool = ctx.enter_context(tc.tile_pool(name="x", bufs=1))
        wupool = ctx.enter_context(tc.tile_pool(name="wu", bufs=1))
        # one sq pair-buffer per (ng, pair): no slot reuse → no WAR wait
        sqpool = ctx.enter_context(tc.tile_pool(name="sq", bufs=NGRP * 2))
        cppool = ctx.enter_context(tc.tile_pool(name="cp", bufs=4))
        rowpool = ctx.enter_context(tc.tile_pool(name="row", bufs=1))
        pspool = ctx.enter_context(tc.tile_pool(name="ps", bufs=4, space="PSUM"))
        wtpool = ctx.enter_context(tc.tile_pool(name="wt", bufs=1, space="PSUM"))
        psepool = ctx.enter_context(tc.tile_pool(name="pse", bufs=1, space="PSUM"))

        xgsb = xpool.tile([P, COLS], FP8)

        # PE warm-up: keep the HAM un-throttled while the input DMA runs.
        dmy = wupool.tile([P, 256], BF16)
        nc.gpsimd.memset(dmy[:, :], 0.0)
        wt = wtpool.tile([P, 256], F32, tag="wt", name="wt")
        for w in range(NWARM):
            nc.tensor.matmul(
                wt[:, :], dmy[:, :P], dmy[:, :],
                start=(w == 0), stop=(w == NWARM - 1),
            )

        # chunked input DMA: weights+selector first, then x in 4 chunks
        nc.sync.dma_start(out=xgsb[:, :XBASE], in_=xg[:, :XBASE])
        for k in range(NCHUNK):
            c0 = XBASE + k * CHUNK_COLS
            nc.sync.dma_start(
                out=xgsb[:, c0 : c0 + CHUNK_COLS], in_=xg[:, c0 : c0 + CHUNK_COLS]
            )

        # prefence: a standalone PE weight-load per DMA absorbs that DMA's
        # semaphore wait, so every real matmul carries at most one wait
        nc.tensor.ldweights(xgsb[:, :P])

        # per-pair PSUM energy accumulators [2, 512]
        pes = [
            psepool.tile([2, 512], F32, tag=f"pe{pr}", name=f"pe{pr}")
            for pr in range(2)
        ]

        for ng in range(NGRP):
            k = ng // 4
            base = XBASE + k * CHUNK_COLS + (ng % 4) * 4 * P
            if ng % 4 == 0:
                nc.tensor.ldweights(xgsb[:, base : base + P])  # chunk prefence
            for sl in range(S_PER):
                gbase = sl * 3 * P
                pt = pspool.tile([P, 512], F32)
                # DoubleRow conv: D=0,1 fused (K=256)
                lhsT_dr = _strided(xgsb[:, gbase : gbase + P], [[P, 2], [1, P]])
                rhs_dr = _strided(xgsb[:, base : base + 512], [[P, 2], [1, 512]])
                nc.tensor.matmul(
                    pt[:, :], lhsT_dr, rhs_dr, start=True, stop=False, perf_mode=DR
                )
                # D=2 normal fp8 matmul
                nc.tensor.matmul(
                    pt[:, :],
                    xgsb[:, gbase + 2 * P : gbase + 3 * P],
                    xgsb[:, base + 2 * P : base + 2 * P + 512],
                    start=False, stop=True,
                )
                pr, half = sl // 2, sl % 2
                if half == 0:
                    sqp = sqpool.tile([P, 1024], FP8)
                    if ng == 0 and pr == 0:
                        sqtiles = {}
                    sqtiles[pr] = sqp
                else:
                    sqp = sqtiles[pr]
                dst = sqp[:, half * 512 : half * 512 + 512]
                if pr == 0:
                    # ACT: square+cast straight out of PSUM
                    nc.scalar.square(dst, pt[:, :])
                else:
                    # DVE: bf16 copy out of PSUM, then 2x bf16 square
                    cp = cppool.tile([P, 512], BF16)
                    nc.vector.tensor_copy(cp[:, :], pt[:, :])
                    nc.vector.tensor_mul(dst, cp[:, :], cp[:, :])
                if half == 1:
                    # DoubleRow selector reduce: both scales of the pair at once
                    lhsT_red = _strided(
                        xgsb[:, SELBASE : SELBASE + 2], [[16, 2], [1, 2]]
                    )
                    rhs_red = _strided(sqp[:, 0:512], [[512, 2], [1, 512]])
                    nc.tensor.matmul(
                        pes[pr][:, :], lhsT_red, rhs_red,
                        start=(ng == 0), stop=(ng == NGRP - 1), perf_mode=DR,
                    )

        # tail: evict the two [2,512] accumulators in parallel, two DMAs
        rowout = rowpool.tile([2, 1024], F32, tag="rowout", name="rowout")
        nc.scalar.copy(rowout[:, 0:512], pes[0][:, :])
        nc.vector.tensor_copy(rowout[:, 512:1024], pes[1][:, :])
        nc.sync.dma_start(out=outp[:, 0:512], in_=rowout[:, 0:512])
        nc.sync.dma_start(out=outp[:, 512:1024], in_=rowout[:, 512:1024])

    return nc


_NC_CACHE = None


def _get_nc() -> bass.Bass:
    global _NC_CACHE
    if _NC_CACHE is None:
        _NC_CACHE = _build_nc()
    return _NC_CACHE


def kernel(x: np.ndarray, scale_weights: np.ndarray, _trace: bool = False) -> np.ndarray:
    global LAST_RESULTS
    x = np.asarray(x, dtype=np.float32)
    scale_weights = np.asarray(scale_weights, dtype=np.float32)
    assert x.shape == (P, NT) and scale_weights.shape == (S_TOTAL,)

    import ml_dtypes

    fp8 = ml_dtypes.float8_e4m3fn

    # host prep: zero-pad, transpose to time-major blocked layout
    xpad = np.zeros((NBLK * P, P), dtype=np.float32)
    xpad[P : P + NT, :] = x.T
    # xb2[a, A*128 + b] = xpad[A*128 + a, b]
    xb2 = np.ascontiguousarray(
        xpad.reshape(NBLK, P, P).transpose(1, 0, 2).reshape(P, NBLK * P)
    )
    # 4 chunks of 18 blocks: chunk k = global blocks [16k, 16k+18)
    xch = np.concatenate(
        [xb2[:, 16 * k * P : (16 * k + CHUNK_BLKS) * P] for k in range(NCHUNK)],
        axis=1,
    )

    # DoubleRow reduce selector [128, 32]:
    # cols 0,1 = ko0 weights (scale A -> row 0), cols 16,17 = ko1 (scale B)
    sel = np.zeros((P, 32), dtype=np.float32)
    sel[:, 0] = 1.0
    sel[:, 17] = 1.0

    G = _toeplitz_weights()  # [32, 3, 128, 128]
    xgs = []
    for c in range(N_CORES):
        Gc = G[c * S_PER : (c + 1) * S_PER].reshape(S_PER * 3, P, P)
        gw2 = Gc.transpose(1, 0, 2).reshape(P, GCOLS)
        xgs.append(
            np.ascontiguousarray(
                np.concatenate([gw2, sel, xch], axis=1).astype(fp8)
            )
        )

    nc = _get_nc()
    in_maps = [{"xg": xgs[c]} for c in range(N_CORES)]
    res = run_bass_kernel_spmd(nc, in_maps, list(range(N_CORES)), trace=_trace)
    LAST_RESULTS = res

    # gather + unshard: outp[c] is [2, (pair, Bsub, b)] f32;
    # scale 4c + 2*pair + row, energy sum = fold Bsub
    esum = np.zeros((S_TOTAL, P), dtype=np.float64)
    for c in range(N_CORES):
        arr = np.asarray(res.results[c]["outp"], dtype=np.float64)  # [2, 1024]
        for pr in range(2):
            blk = arr[:, pr * 512 : (pr + 1) * 512].reshape(2, 4, P).sum(axis=1)
            esum[c * S_PER + 2 * pr + 0] = blk[0]
            esum[c * S_PER + 2 * pr + 1] = blk[1]
    energy = (esum.T / np.float64(NT)).astype(np.float32)

    w = scale_weights.astype(np.float64)
    e = np.exp(w - w.max())
    sm = (e / e.sum()).astype(np.float32)
    return (energy * sm[None, :]).astype(np.float32)


if __name__ == "__main__":
    rng = np.random.default_rng(0)
    x = rng.standard_normal((P, NT), dtype=np.float32)
    sw = rng.standard_normal(S_TOTAL, dtype=np.float32)
    out = kernel(x, sw)
    print("kernel output shape:", out.shape, out.dtype)


# revision 6
# speedup vs baseline: 1.8978x; 1.0697x over previous
"""Trainium2 Bass kernel for ContinuousWaveletLayer (CWT energy).

Reference computation:
  bank = Morlet wavelet bank [32 scales, Lmax=256] (static)
  coef[b,s,t] = 'same' conv of x[b,:] (len 8192) with bank[s,:]
  out[b,s]    = mean_t(coef^2) * softmax(scale_weights)[s]

Device strategy (8 NeuronCores, scale-parallel, 4 scales/core):
  Toeplitz matmuls on the tensor engine in fp8 (e4m3).  With x padded
  (128/128) and viewed time-major in blocks of 128, output block B for
  scale s is   coef_B = sum_{D=0..2} G[s,D].T @ Xblk[B+D]  (K=128).
  The D=0,1 pair is fused into one DoubleRow matmul (K=256, 2 fp8
  weights per PE cell), D=2 is a normal fp8 matmul accumulating into
  the same PSUM bank.  Squares evacuate PSUM on ACT (scales 0,1 ->
  fp8 direct) and DVE (scales 2,3: bf16 copy + 2x bf16 mul -> fp8).
  Per-scale time-reduction is a DoubleRow "selector" matmul that
  reduces TWO scales' squares at once into a [2,512] PSUM accumulator
  (K=256, accumulated over all 16 block groups).  The input DMA is
  split into 5 chunks so conv starts ~1.5us after the DMA begins, and
  dummy warm-up matmuls keep the PE HAM un-throttled during the DMA.
  Host folds the 4 sub-block columns, applies 1/N and softmax.
"""

import os
import sys
from contextlib import ExitStack

import numpy as np

sys.path.insert(0, "/opt/trn_rl_repo")

import concourse.bass as bass
import concourse.mybir as mybir
from concourse import tile
from concourse.ap import AP
from concourse.bass_utils import run_bass_kernel_spmd
from concourse.vector_clock import ScopedClock


def _drain_and_barrier_single_wait(self, tick_clock, wait_clock):
    """TileContext._drain_and_barrier, but the kernel-tail drain's
    global-clock waits are spread over a chain of single-wait drains —
    the walrus build here allows only one sync wait per instruction."""
    drain_inst = self.nc.sync.drain()
    wait_clock.add_sem_waits(
        drain_inst.ins, ScopedClock({None: tick_clock.global_clock})
    )
    si = drain_inst.ins.sync_info
    waits = list(si.on_wait)
    if len(waits) > 1:
        si.on_wait = [waits[0]]
        sems = {h.name: h for h in self.sems.allocated().values()}
        for w in waits[1:]:
            d2 = self.nc.sync.drain()
            d2.wait_op(sems[w.ant_name], w.wait_value, "sem-ge")
    self.nc.all_engine_barrier()
    assert self.sems is not None
    popped = self.nc._tile_sem_poison_stack.pop()
    assert popped is self._sem_poison
    self.nc.clear_and_free_semaphores(list(self.sems.allocated().values()))
    self.nc.all_engine_barrier()


tile.TileContext._drain_and_barrier = _drain_and_barrier_single_wait

N_CORES = 8
S_TOTAL = 32          # number of scales
S_PER = 4             # scales per core
P = 128               # partition / block size
NT = 8192             # time samples
LMAX = 256            # padded kernel length
NBLK = 66             # input blocks: (128 + 8192 + 128) / 128
NGRP = 16             # groups of 4 output blocks (N=512 matmuls)
F32 = mybir.dt.float32
BF16 = mybir.dt.bfloat16
FP8 = mybir.dt.float8e4
DR = mybir.MatmulPerfMode.DoubleRow

GCOLS = S_PER * 3 * P          # 1536 weight columns
SELBASE = GCOLS                # DoubleRow reduce selector (32 cols)
XBASE = GCOLS + 32
CHUNK_BLKS = 18                # x chunk k holds blocks [16k, 16k+18)
CHUNK_COLS = CHUNK_BLKS * P    # 2304
NCHUNK = 4
COLS = XBASE + NCHUNK * CHUNK_COLS
NWARM = 18                     # PE warm-up matmuls during the input DMA

LAST_RESULTS = None   # BassKernelResults of the most recent run (for test.py)


def _morlet_kernel_bank(n_scales: int, n: int) -> np.ndarray:
    Lmax = min(8 * n_scales, n)
    bank = np.zeros((n_scales, Lmax), dtype=np.float32)
    for i, s in enumerate(range(1, n_scales + 1)):
        L = min(8 * s, n)
        t = np.linspace(-4.0 * s, 4.0 * s, L)
        w = np.exp(-t**2 / (2.0 * s**2)) * np.cos(5.0 * t / s)
        w = w / np.sqrt(s)
        off = (Lmax - 1) // 2 - (L - 1) // 2
        bank[i, off : off + L] = w.astype(np.float32)
    return bank


def _toeplitz_weights() -> np.ndarray:
    """G[s, D][a, to] = g_s[128*D + a - to], zero outside support."""
    bank = _morlet_kernel_bank(S_TOTAL, NT)          # [32, 256]
    g = bank[:, ::-1].copy()                         # reversed rows
    a = np.arange(P)[:, None]
    to = np.arange(P)[None, :]
    G = np.zeros((S_TOTAL, 3, P, P), dtype=np.float32)
    for D in range(3):
        d = 128 * D + a - to
        valid = (d >= 0) & (d < LMAX)
        dc = np.clip(d, 0, LMAX - 1)
        for s in range(S_TOTAL):
            G[s, D] = np.where(valid, g[s][dc], 0.0)
    return G


def _strided(sl, dims):
    """Manual AP on a tile slice: dims = [(stride, n), ...] free dims,
    partition dim inherited from the 2D slice."""
    return AP(
        tensor=sl.tensor,
        offset=sl.offset,
        ap=[list(sl.ap[0])] + [[st, n] for st, n in dims],
    )


def _build_nc() -> bass.Bass:
    nc = bass.Bass()
    xg = nc.dram_tensor("xg", [P, COLS], FP8, kind="ExternalInput")
    # per-core partial energies: [2, (pair, Bsub, b)]; scale = 2*pair + row
    outp = nc.dram_tensor("outp", [2, 1024], F32, kind="ExternalOutput")

    with tile.TileContext(nc) as tc, ExitStack() as ctx:
        xpool = ctx.enter_context(tc.tile_pool(name="x", bufs=1))
        wupool = ctx.enter_context(tc.tile_pool(name="wu", bufs=1))
        # one sq pair-buffer per (ng, pair): no slot reuse → no WAR wait
        sqpool = ctx.enter_context(tc.tile_pool(name="sq", bufs=NGRP * 2))
        cppool = ctx.enter_context(tc.tile_pool(name="cp", bufs=4))
        rowpool = ctx.enter_context(tc.tile_pool(name="row", bufs=1))
        pspool = ctx.enter_context(tc.tile_pool(name="ps", bufs=4, space="PSUM"))
        wtpool = ctx.enter_context(tc.tile_pool(name="wt", bufs=1, space="PSUM"))
        psepool = ctx.enter_context(tc.tile_pool(name="pse", bufs=1, space="PSUM"))

        xgsb = xpool.tile([P, COLS], FP8)

        # PE warm-up: keep the HAM un-throttled while the input DMA runs.
        dmy = wupool.tile([P, 256], BF16)
        nc.gpsimd.memset(dmy[:, :], 0.0)
        wt = wtpool.tile([P, 256], F32, tag="wt", name="wt")
        for w in range(NWARM):
            nc.tensor.matmul(
                wt[:, :], dmy[:, :P], dmy[:, :],
                start=(w == 0), stop=(w == NWARM - 1),
            )

        # chunked input DMA: weights+selector first, then x in 4 chunks
        nc.sync.dma_start(out=xgsb[:, :XBASE], in_=xg[:, :XBASE])
        for k in range(NCHUNK):
            c0 = XBASE + k * CHUNK_COLS
            nc.sync.dma_start(
                out=xgsb[:, c0 : c0 + CHUNK_COLS], in_=xg[:, c0 : c0 + CHUNK_COLS]
            )

        # prefence: a standalone PE weight-load per DMA absorbs that DMA's
        # semaphore wait, so every real matmul carries at most one wait
        nc.tensor.ldweights(xgsb[:, :P])

        # per-pair PSUM energy accumulators [2, 512]
        pes = [
            psepool.tile([2, 512], F32, tag=f"pe{pr}", name=f"pe{pr}")
            for pr in range(2)
        ]

        for ng in range(NGRP):
            k = ng // 4
            base = XBASE + k * CHUNK_COLS + (ng % 4) * 4 * P
            if ng % 4 == 0:
                nc.tensor.ldweights(xgsb[:, base : base + P])  # chunk prefence
            for sl in range(S_PER):
                gbase = sl * 3 * P
                pt = pspool.tile([P, 512], F32)
                # DoubleRow conv: D=0,1 fused (K=256)
                lhsT_dr = _strided(xgsb[:, gbase : gbase + P], [[P, 2], [1, P]])
                rhs_dr = _strided(xgsb[:, base : base + 512], [[P, 2], [1, 512]])
                nc.tensor.matmul(
                    pt[:, :], lhsT_dr, rhs_dr, start=True, stop=False, perf_mode=DR
                )
                # D=2 normal fp8 matmul
                nc.tensor.matmul(
                    pt[:, :],
                    xgsb[:, gbase + 2 * P : gbase + 3 * P],
                    xgsb[:, base + 2 * P : base + 2 * P + 512],
                    start=False, stop=True,
                )
                pr, half = sl // 2, sl % 2
                if half == 0:
                    sqp = sqpool.tile([P, 1024], FP8)
                    if ng == 0 and pr == 0:
                        sqtiles = {}
                    sqtiles[pr] = sqp
                else:
                    sqp = sqtiles[pr]
                dst = sqp[:, half * 512 : half * 512 + 512]
                # ACT squares 3 of 4 scales (it is the faster PSUM-evict
                # path); DVE takes the 4th, except on the last group where
                # ACT takes all 4 to shorten the tail chain
                if sl < 3 or ng == NGRP - 1:
                    # ACT: square+cast straight out of PSUM
                    nc.scalar.square(dst, pt[:, :])
                else:
                    # DVE: bf16 copy out of PSUM, then bf16 square
                    cp = cppool.tile([P, 512], BF16)
                    nc.vector.tensor_copy(cp[:, :], pt[:, :])
                    nc.vector.tensor_mul(dst, cp[:, :], cp[:, :])
                if half == 1:
                    # DoubleRow selector reduce: both scales of the pair at once
                    lhsT_red = _strided(
                        xgsb[:, SELBASE : SELBASE + 2], [[16, 2], [1, 2]]
                    )
                    rhs_red = _strided(sqp[:, 0:512], [[512, 2], [1, 512]])
                    if sl == 3 and ng < NGRP - 1:
                        # fence: absorb the DVE wait so the mixed-writer
                        # pair's reduce carries a single sync wait
                        nc.tensor.ldweights(sqp[:, 512 : 512 + P])
                    nc.tensor.matmul(
                        pes[pr][:, :], lhsT_red, rhs_red,
                        start=(ng == 0), stop=(ng == NGRP - 1), perf_mode=DR,
                    )

        # tail: evict the two [2,512] accumulators in parallel, two DMAs
        rowout = rowpool.tile([2, 1024], F32, tag="rowout", name="rowout")
        nc.scalar.copy(rowout[:, 0:512], pes[0][:, :])
        nc.vector.tensor_copy(rowout[:, 512:1024], pes[1][:, :])
        nc.sync.dma_start(out=outp[:, 0:512], in_=rowout[:, 0:512])
        nc.sync.dma_start(out=outp[:, 512:1024], in_=rowout[:, 512:1024])

    return nc


_NC_CACHE = None


def _get_nc() -> bass.Bass:
    global _NC_CACHE
    if _NC_CACHE is None:
        _NC_CACHE = _build_nc()
    return _NC_CACHE


def kernel(x: np.ndarray, scale_weights: np.ndarray, _trace: bool = False) -> np.ndarray:
    global LAST_RESULTS
    x = np.asarray(x, dtype=np.float32)
    scale_weights = np.asarray(scale_weights, dtype=np.float32)
    assert x.shape == (P, NT) and scale_weights.shape == (S_TOTAL,)

    import ml_dtypes

    fp8 = ml_dtypes.float8_e4m3fn

    # host prep: zero-pad, transpose to time-major blocked layout
    xpad = np.zeros((NBLK * P, P), dtype=np.float32)
    xpad[P : P + NT, :] = x.T
    # xb2[a, A*128 + b] = xpad[A*128 + a, b]
    xb2 = np.ascontiguousarray(
        xpad.reshape(NBLK, P, P).transpose(1, 0, 2).reshape(P, NBLK * P)
    )
    # 4 chunks of 18 blocks: chunk k = global blocks [16k, 16k+18)
    xch = np.concatenate(
        [xb2[:, 16 * k * P : (16 * k + CHUNK_BLKS) * P] for k in range(NCHUNK)],
        axis=1,
    )

    # DoubleRow reduce selector [128, 32]:
    # cols 0,1 = ko0 weights (scale A -> row 0), cols 16,17 = ko1 (scale B)
    sel = np.zeros((P, 32), dtype=np.float32)
    sel[:, 0] = 1.0
    sel[:, 17] = 1.0

    G = _toeplitz_weights()  # [32, 3, 128, 128]
    xgs = []
    for c in range(N_CORES):
        Gc = G[c * S_PER : (c + 1) * S_PER].reshape(S_PER * 3, P, P)
        gw2 = Gc.transpose(1, 0, 2).reshape(P, GCOLS)
        xgs.append(
            np.ascontiguousarray(
                np.concatenate([gw2, sel, xch], axis=1).astype(fp8)
            )
        )

    nc = _get_nc()
    in_maps = [{"xg": xgs[c]} for c in range(N_CORES)]
    res = run_bass_kernel_spmd(nc, in_maps, list(range(N_CORES)), trace=_trace)
    LAST_RESULTS = res

    # gather + unshard: outp[c] is [2, (pair, Bsub, b)] f32;
    # scale 4c + 2*pair + row, energy sum = fold Bsub
    esum = np.zeros((S_TOTAL, P), dtype=np.float64)
    for c in range(N_CORES):
        arr = np.asarray(res.results[c]["outp"], dtype=np.float64)  # [2, 1024]
        for pr in range(2):
            blk = arr[:, pr * 512 : (pr + 1) * 512].reshape(2, 4, P).sum(axis=1)
            esum[c * S_PER + 2 * pr + 0] = blk[0]
            esum[c * S_PER + 2 * pr + 1] = blk[1]
    energy = (esum.T / np.float64(NT)).astype(np.float32)

    w = scale_weights.astype(np.float64)
    e = np.exp(w - w.max())
    sm = (e / e.sum()).astype(np.float32)
    return (energy * sm[None, :]).astype(np.float32)


if __name__ == "__main__":
    rng = np.random.default_rng(0)
    x = rng.standard_normal((P, NT), dtype=np.float32)
    sw = rng.standard_normal(S_TOTAL, dtype=np.float32)
    out = kernel(x, sw)
    print("kernel output shape:", out.shape, out.dtype)


# revision 10
# speedup vs baseline: 1.9370x; 1.0207x over previous
"""Trainium2 Bass kernel for ContinuousWaveletLayer (CWT energy).

Reference computation:
  bank = Morlet wavelet bank [32 scales, Lmax=256] (static)
  coef[b,s,t] = 'same' conv of x[b,:] (len 8192) with bank[s,:]
  out[b,s]    = mean_t(coef^2) * softmax(scale_weights)[s]

Device strategy (8 NeuronCores, scale-parallel, 4 scales/core):
  Toeplitz matmuls on the tensor engine in fp8 (e4m3).  Each core gets
  2 "small" scales (L=8s <= 128) and 2 "large" ones.  Small scales are
  realigned per scale (their own time-shifted copy of x) so one output
  block needs a 2-block input window -> ONE DoubleRow matmul (K=256,
  2 fp8 weights per PE cell).  Large scales use the centered layout:
  a DoubleRow matmul for the D=0,1 pair plus a normal fp8 matmul for
  D=2, accumulated in PSUM.  Squares evacuate PSUM on ACT (3 of 4
  scales -> fp8 direct) and DVE (bf16 copy + bf16 mul).  Per-scale
  time-reduction is a DoubleRow "selector" matmul reducing TWO scales'
  squares at once into a [2,512] PSUM accumulator (K=256, accumulated
  over all 16 block groups).  The input DMA is split into 13 chunks so
  conv starts ~3us after the DMA begins; dummy warm-up matmuls keep
  the PE HAM un-throttled during the DMA; standalone weight-loads
  "prefence" each DMA/cross-engine dependency so every matmul carries
  at most one sync wait.  Host folds the 4 sub-block columns, applies
  1/N and softmax.
"""

import os
import sys
from contextlib import ExitStack

import numpy as np

sys.path.insert(0, "/opt/trn_rl_repo")

import concourse.bass as bass
import concourse.mybir as mybir
from concourse import tile
from concourse.ap import AP
from concourse.bass_utils import run_bass_kernel_spmd
from concourse.vector_clock import ScopedClock


def _drain_and_barrier_single_wait(self, tick_clock, wait_clock):
    """TileContext._drain_and_barrier, but the kernel-tail drain's
    global-clock waits are spread over a chain of single-wait drains —
    the walrus build here allows only one sync wait per instruction."""
    drain_inst = self.nc.sync.drain()
    wait_clock.add_sem_waits(
        drain_inst.ins, ScopedClock({None: tick_clock.global_clock})
    )
    si = drain_inst.ins.sync_info
    waits = list(si.on_wait)
    if len(waits) > 1:
        si.on_wait = [waits[0]]
        sems = {h.name: h for h in self.sems.allocated().values()}
        for w in waits[1:]:
            d2 = self.nc.sync.drain()
            d2.wait_op(sems[w.ant_name], w.wait_value, "sem-ge")
    self.nc.all_engine_barrier()
    assert self.sems is not None
    popped = self.nc._tile_sem_poison_stack.pop()
    assert popped is self._sem_poison
    self.nc.clear_and_free_semaphores(list(self.sems.allocated().values()))
    self.nc.all_engine_barrier()


tile.TileContext._drain_and_barrier = _drain_and_barrier_single_wait

N_CORES = 8
S_TOTAL = 32          # number of scales
S_PER = 4             # scales per core
P = 128               # partition / block size
NT = 8192             # time samples
LMAX = 256            # padded kernel length
NBLK = 66             # input blocks: (128 + 8192 + 128) / 128
NGRP = 16             # groups of 4 output blocks (N=512 matmuls)
F32 = mybir.dt.float32
BF16 = mybir.dt.bfloat16
FP8 = mybir.dt.float8e4
DR = mybir.MatmulPerfMode.DoubleRow

# slot -> (g col base, x array, n D-blocks); slots 0,1 small; 2,3 large
SLOTCFG = [(0, 0, 2), (256, 1, 2), (512, 2, 3), (896, 2, 3)]
GCOLS = 1280
SELBASE = GCOLS                # DoubleRow reduce selector (32 cols)
XBASE = GCOLS + 32
CHUNK_BLKS = 18                # x chunk k holds blocks [16k, 16k+18)
CHUNK_COLS = CHUNK_BLKS * P    # 2304
NCHUNK = 4
XARR = NCHUNK * CHUNK_COLS     # 9216 cols per x array (xsA, xsB, xL)
COLS = XBASE + 3 * XARR
NWARM = 18                     # PE warm-up matmuls during the input DMA

LAST_RESULTS = None   # BassKernelResults of the most recent run (for test.py)


def _morlet_kernel_bank(n_scales: int, n: int) -> np.ndarray:
    Lmax = min(8 * n_scales, n)
    bank = np.zeros((n_scales, Lmax), dtype=np.float32)
    for i, s in enumerate(range(1, n_scales + 1)):
        L = min(8 * s, n)
        t = np.linspace(-4.0 * s, 4.0 * s, L)
        w = np.exp(-t**2 / (2.0 * s**2)) * np.cos(5.0 * t / s)
        w = w / np.sqrt(s)
        off = (Lmax - 1) // 2 - (L - 1) // 2
        bank[i, off : off + L] = w.astype(np.float32)
    return bank


_A = np.arange(P)[:, None]
_TO = np.arange(P)[None, :]


def _toeplitz(gtaps: np.ndarray, ndblk: int) -> np.ndarray:
    """[ndblk, 128, 128] blocks: G[D][a,to] = gtaps[128*D + a - to]."""
    L = len(gtaps)
    out = np.zeros((ndblk, P, P), dtype=np.float32)
    for D in range(ndblk):
        d = 128 * D + _A - _TO
        valid = (d >= 0) & (d < L)
        out[D] = np.where(valid, gtaps[np.clip(d, 0, L - 1)], 0.0)
    return out


def _core_scales(c: int) -> list[int]:
    """Global 0-based scale indices for core c's 4 slots."""
    return [2 * c, 2 * c + 1, 16 + 2 * c, 17 + 2 * c]


def _strided(sl, dims):
    """Manual AP on a tile slice: dims = [(stride, n), ...] free dims,
    partition dim inherited from the 2D slice."""
    return AP(
        tensor=sl.tensor,
        offset=sl.offset,
        ap=[list(sl.ap[0])] + [[st, n] for st, n in dims],
    )


def _build_nc() -> bass.Bass:
    nc = bass.Bass()
    xg = nc.dram_tensor("xg", [P, COLS], FP8, kind="ExternalInput")
    # per-core partial energies: [2, (pair, Bsub, b)]; slot = 2*pair + row
    outp = nc.dram_tensor("outp", [2, 1024], F32, kind="ExternalOutput")

    with tile.TileContext(nc) as tc, ExitStack() as ctx:
        xpool = ctx.enter_context(tc.tile_pool(name="x", bufs=1))
        wupool = ctx.enter_context(tc.tile_pool(name="wu", bufs=1))
        sqpool = ctx.enter_context(tc.tile_pool(name="sq", bufs=NGRP * 2))
        cppool = ctx.enter_context(tc.tile_pool(name="cp", bufs=4))
        rowpool = ctx.enter_context(tc.tile_pool(name="row", bufs=1))
        pspool = ctx.enter_context(tc.tile_pool(name="ps", bufs=5, space="PSUM"))
        wtpool = ctx.enter_context(tc.tile_pool(name="wt", bufs=1, space="PSUM"))
        psepool = ctx.enter_context(tc.tile_pool(name="pse", bufs=1, space="PSUM"))

        xgsb = xpool.tile([P, COLS], FP8)

        # PE warm-up: keep the HAM un-throttled while the input DMA runs.
        dmy = wupool.tile([P, 256], BF16)
        nc.gpsimd.memset(dmy[:, :], 0.0)
        wt = wtpool.tile([P, 256], F32, tag="wt", name="wt")
        for w in range(NWARM):
            nc.tensor.matmul(
                wt[:, :], dmy[:, :P], dmy[:, :],
                start=(w == 0), stop=(w == NWARM - 1),
            )

        # chunked input DMA: weights+selector first, then the three x
        # arrays chunk-tier by chunk-tier
        nc.sync.dma_start(out=xgsb[:, :XBASE], in_=xg[:, :XBASE])
        for k in range(NCHUNK):
            for arr in range(3):
                c0 = XBASE + arr * XARR + k * CHUNK_COLS
                nc.sync.dma_start(
                    out=xgsb[:, c0 : c0 + CHUNK_COLS],
                    in_=xg[:, c0 : c0 + CHUNK_COLS],
                )

        # prefence the weights DMA
        nc.tensor.ldweights(xgsb[:, :P])

        # per-pair PSUM energy accumulators [2, 512]
        pes = [
            psepool.tile([2, 512], F32, tag=f"pe{pr}", name=f"pe{pr}")
            for pr in range(2)
        ]

        for ng in range(NGRP):
            k = ng // 4
            loc = (ng % 4) * 4 * P
            if ng % 4 == 0:
                for arr in range(3):  # chunk-tier prefences
                    c0 = XBASE + arr * XARR + k * CHUNK_COLS
                    nc.tensor.ldweights(xgsb[:, c0 : c0 + P])
            # ACT evacuates 3 of 4 PSUM tiles (it is faster per tile);
            # every 4th group DVE takes two to balance totals
            dve_slots = {2, 3} if (ng % 4 == 3 and ng != NGRP - 1) else {3}
            sqtiles = {}
            for sl in range(S_PER):
                gbase, arr, nd = SLOTCFG[sl]
                base = XBASE + arr * XARR + k * CHUNK_COLS + loc
                pt = pspool.tile([P, 512], F32)
                # DoubleRow conv: D=0,1 fused (K=256)
                lhsT_dr = _strided(xgsb[:, gbase : gbase + P], [[P, 2], [1, P]])
                rhs_dr = _strided(xgsb[:, base : base + 512], [[P, 2], [1, 512]])
                nc.tensor.matmul(
                    pt[:, :], lhsT_dr, rhs_dr,
                    start=True, stop=(nd == 2), perf_mode=DR,
                )
                if nd == 3:  # large scale: normal fp8 matmul for D=2
                    nc.tensor.matmul(
                        pt[:, :],
                        xgsb[:, gbase + 2 * P : gbase + 3 * P],
                        xgsb[:, base + 2 * P : base + 2 * P + 512],
                        start=False, stop=True,
                    )
                pr, half = sl // 2, sl % 2
                if half == 0:
                    sqp = sqpool.tile([P, 1024], FP8)
                    sqtiles[pr] = sqp
                else:
                    sqp = sqtiles[pr]
                dst = sqp[:, half * 512 : half * 512 + 512]
                if sl not in dve_slots:
                    # ACT: square+cast straight out of PSUM
                    nc.scalar.square(dst, pt[:, :])
                else:
                    # DVE: bf16 copy out of PSUM, then bf16 square
                    cp = cppool.tile([P, 512], BF16)
                    nc.vector.tensor_copy(cp[:, :], pt[:, :])
                    nc.vector.tensor_mul(dst, cp[:, :], cp[:, :])
                if half == 1:
                    # DoubleRow selector reduce: both scales of the pair
                    lhsT_red = _strided(
                        xgsb[:, SELBASE : SELBASE + 2], [[16, 2], [1, 2]]
                    )
                    rhs_red = _strided(sqp[:, 0:512], [[512, 2], [1, 512]])
                    if pr == 1 and dve_slots == {3}:
                        # fence: absorb the DVE wait so the mixed-writer
                        # pair's reduce carries a single sync wait
                        nc.tensor.ldweights(sqp[:, 512 : 512 + P])
                    nc.tensor.matmul(
                        pes[pr][:, :], lhsT_red, rhs_red,
                        start=(ng == 0), stop=(ng == NGRP - 1), perf_mode=DR,
                    )

        # tail: evict the two [2,512] accumulators in parallel, two DMAs
        rowout = rowpool.tile([2, 1024], F32, tag="rowout", name="rowout")
        nc.scalar.copy(rowout[:, 0:512], pes[0][:, :])
        nc.vector.tensor_copy(rowout[:, 512:1024], pes[1][:, :])
        nc.gpsimd.dma_start(out=outp[:, 0:512], in_=rowout[:, 0:512])
        nc.gpsimd.dma_start(out=outp[:, 512:1024], in_=rowout[:, 512:1024])

    return nc


_NC_CACHE = None


def _get_nc() -> bass.Bass:
    global _NC_CACHE
    if _NC_CACHE is None:
        _NC_CACHE = _build_nc()
    return _NC_CACHE


def _block_chunks(xs2: np.ndarray) -> np.ndarray:
    """[NBLK*P, P] time-major -> [P, XARR] blocked, 4 chunks of 18 blocks."""
    xb2 = np.ascontiguousarray(
        xs2.reshape(NBLK, P, P).transpose(1, 0, 2).reshape(P, NBLK * P)
    )
    return np.concatenate(
        [xb2[:, 16 * k * P : (16 * k + CHUNK_BLKS) * P] for k in range(NCHUNK)],
        axis=1,
    )


def kernel(x: np.ndarray, scale_weights: np.ndarray, _trace: bool = False) -> np.ndarray:
    global LAST_RESULTS
    x = np.asarray(x, dtype=np.float32)
    scale_weights = np.asarray(scale_weights, dtype=np.float32)
    assert x.shape == (P, NT) and scale_weights.shape == (S_TOTAL,)

    import ml_dtypes

    fp8 = ml_dtypes.float8_e4m3fn

    bank = _morlet_kernel_bank(S_TOTAL, NT)     # [32, 256]
    grev = bank[:, ::-1].copy()                 # reversed rows

    # host prep: zero-pad, per-shift time-major blocked layouts
    xpad = np.zeros((NBLK * P, P), dtype=np.float32)
    xpad[P : P + NT, :] = x.T
    xchL = _block_chunks(xpad)

    # DoubleRow reduce selector [128, 32]:
    # cols 0,1 = ko0 weights (slot A -> row 0), cols 16,17 = ko1 (slot B)
    sel = np.zeros((P, 32), dtype=np.float32)
    sel[:, 0] = 1.0
    sel[:, 17] = 1.0

    xgs = []
    for c in range(N_CORES):
        scales = _core_scales(c)
        gw = np.zeros((P, GCOLS), dtype=np.float32)
        xarrs = []
        for sl in range(S_PER):
            gbase, arr, nd = SLOTCFG[sl]
            sidx = scales[sl]
            if nd == 2:  # small scale: realigned taps + shifted x copy
                s = sidx + 1
                off, L = 128 - 4 * s, 8 * s
                gtaps = grev[sidx][off : off + L]
                xs2 = np.zeros_like(xpad)
                xs2[: NBLK * P - off, :] = xpad[off:, :]
                xarrs.append(_block_chunks(xs2))
            else:
                gtaps = grev[sidx]
            G = _toeplitz(np.asarray(gtaps, dtype=np.float64), nd)
            for D in range(nd):
                gw[:, gbase + D * P : gbase + (D + 1) * P] = G[D]
        xarrs.append(xchL)
        xgs.append(
            np.ascontiguousarray(
                np.concatenate([gw, sel] + xarrs, axis=1).astype(fp8)
            )
        )

    nc = _get_nc()
    in_maps = [{"xg": xgs[c]} for c in range(N_CORES)]
    res = run_bass_kernel_spmd(nc, in_maps, list(range(N_CORES)), trace=_trace)
    LAST_RESULTS = res

    # gather + unshard: outp[c] is [2, (pair, Bsub, b)] f32; slot = 2pr+row
    esum = np.zeros((S_TOTAL, P), dtype=np.float64)
    for c in range(N_CORES):
        scales = _core_scales(c)
        arr = np.asarray(res.results[c]["outp"], dtype=np.float64)  # [2, 1024]
        for pr in range(2):
            blk = arr[:, pr * 512 : (pr + 1) * 512].reshape(2, 4, P).sum(axis=1)
            esum[scales[2 * pr + 0]] = blk[0]
            esum[scales[2 * pr + 1]] = blk[1]
    energy = (esum.T / np.float64(NT)).astype(np.float32)

    w = scale_weights.astype(np.float64)
    e = np.exp(w - w.max())
    sm = (e / e.sum()).astype(np.float32)
    return (energy * sm[None, :]).astype(np.float32)


if __name__ == "__main__":
    rng = np.random.default_rng(0)
    x = rng.standard_normal((P, NT), dtype=np.float32)
    sw = rng.standard_normal(S_TOTAL, dtype=np.float32)
    out = kernel(x, sw)
    print("kernel output shape:", out.shape, out.dtype)
